# revision 11
# baseline (speedup 1.0000x reference)
"""Trainium2 Bass kernel for nn_DotPred (gnn_message_passing).

score[t, e] = sum_d (x[src] - x[dst]) / sqrt(D)
            = (rowsum(x)[src] - rowsum(x)[dst]) / sqrt(D)

Strategy (8 NeuronCores, SPMD): shard the NODE table across cores; each
endpoint reference (a src or dst occurrence of an edge) is handled by the
core owning that node. Each core emits one signed, scaled contribution per
reference; the host unshards with a scatter-add (each edge position gets
its +src and -dst contributions, possibly from different cores) —
mirroring the all-reduce combine in the sharding hint.

Per core (nodes [i*NPC, (i+1)*NPC), NPC = 12544):
- Phase 1: rowsum s[n] = sum_d x[n, d] from a bf16 node-slice laid
  [128, 98*128] (node = p*98 + r), via a pairwise bf16 add-tree on DVE
  (2x mode throughout).
- Phase 2 (bucket broadcast): every node gets a FIXED bucket of C slots;
  reference c of node n lives at slot n*C + c. With ~30 refs/node
  (Poisson), C bounds the max (host-asserted on the actual data).
  The output y[p, r, c] = s[p*98+r] * sign[p, r, c] is ONE fused DVE
  tensor_tensor multiply: in0 = s broadcast along c with a stride-0 AP,
  in1 = host-provided sign in {+-1/sqrt(D), 0} (0 pads empty slots).
  No data-dependent addressing on device at all.
"""
import math
from contextlib import ExitStack

import numpy as np

import concourse.bass as bass
import concourse.mybir as mybir
from concourse.bass_utils import run_bass_kernel_spmd

P = 128
D = 128
R = 98                  # s-table columns per partition
NPC = P * R             # nodes per core = 12544
NCORES = 8
C = 64                  # bucket capacity (max refs per node, host-asserted)
INV_SQ = 1.0 / math.sqrt(float(D))
N_NODES = 100000

# embed-load / output chunks: R split as 25+25+24+24 rows
RCH = [25, 25, 24, 24]
RCH_OFF = [0, 25, 50, 74]

F32 = mybir.dt.float32
BF16 = mybir.dt.bfloat16
ALU = mybir.AluOpType


def _build_nc():
    nc = bass.Bass()
    emb = nc.dram_tensor("emb", [P, NPC], BF16, kind="ExternalInput")
    sign_in = nc.dram_tensor("sign_in", [P, R * C], BF16, kind="ExternalInput")
    y = nc.dram_tensor("y", [P, R * C], BF16, kind="ExternalOutput")

    es = ExitStack()
    with es:
        embb0 = es.enter_context(nc.sbuf_tensor("embb0", [P, 25 * D], BF16))
        embb1 = es.enter_context(nc.sbuf_tensor("embb1", [P, 25 * D], BF16))
        embb = [embb0, embb1]
        t64 = es.enter_context(nc.sbuf_tensor([P, R * 64], BF16))
        t16 = es.enter_context(nc.sbuf_tensor([P, R * 16], BF16))
        t4 = es.enter_context(nc.sbuf_tensor([P, R * 4], BF16))
        t2 = es.enter_context(nc.sbuf_tensor([P, R * 2], BF16))
        s_sb = es.enter_context(nc.sbuf_tensor([P, R], BF16))
        sign_sb = es.enter_context(nc.sbuf_tensor([P, R * C], BF16))
        y_sb = es.enter_context(nc.sbuf_tensor([P, R * C], BF16))

        sm_sign = es.enter_context(nc.semaphore())
        sm_embA = es.enter_context(nc.semaphore())
        sm_embB = es.enter_context(nc.semaphore())
        sm_tree = es.enter_context(nc.semaphore())   # level-1 per chunk
        sm_vch = es.enter_context(nc.semaphore())    # vector op chain
        sm_mult = es.enter_context(nc.semaphore())   # per-chunk y ready
        sm_y = es.enter_context(nc.semaphore())
        block = es.enter_context(nc.Block())

        @block.sync
        def _(sync):
            for c in range(4):
                if c >= 2:
                    sync.wait_ge(sm_tree, c - 1)
                sync.dma_start(
                    out=embb[c % 2][:, : RCH[c] * D],
                    in_=emb[:, RCH_OFF[c] * D : (RCH_OFF[c] + RCH[c]) * D],
                ).then_inc([sm_embA, sm_embB][c % 2], 16)
            sync.dma_start(out=sign_sb[:], in_=sign_in[:]).then_inc(sm_sign, 16)
            for c in range(4):
                sync.wait_ge(sm_mult, c + 1)
                sync.dma_start(
                    out=y[:, RCH_OFF[c] * C : (RCH_OFF[c] + RCH[c]) * C],
                    in_=y_sb[:, RCH_OFF[c] * C : (RCH_OFF[c] + RCH[c]) * C],
                ).then_inc(sm_y, 16)

        @block.vector
        def _(vector):
            vch = [0]

            def step(emit):
                emit().then_inc(sm_vch, 1)
                vch[0] += 1
                vector.wait_ge(sm_vch, vch[0])

            # ---- rowsum add-tree (all bf16, 2x) ----
            for c in range(4):
                vector.wait_ge([sm_embA, sm_embB][c % 2], 16 * (c // 2 + 1))
                rc, ro = RCH[c], RCH_OFF[c]
                eb = embb[c % 2]
                vector.tensor_tensor(
                    out=t64[:, ro * 64 : (ro + rc) * 64].rearrange(
                        "p (r d) -> p r d", d=64
                    ),
                    in0=eb[:, : rc * D].rearrange("p (r d) -> p r d", d=D)[:, :, 0:64],
                    in1=eb[:, : rc * D].rearrange("p (r d) -> p r d", d=D)[:, :, 64:128],
                    op=ALU.add,
                ).then_inc(sm_tree, 1)
            vector.wait_ge(sm_tree, 4)
            t64v = t64[:].rearrange("p (r d) -> p r d", d=64)
            t16v = t16[:].rearrange("p (r d) -> p r d", d=16)
            t4v = t4[:].rearrange("p (r d) -> p r d", d=4)
            t2v = t2[:].rearrange("p (r d) -> p r d", d=2)
            step(lambda: vector.tensor_tensor(
                out=t16v, in0=t64v[:, :, 0:16], in1=t64v[:, :, 16:32], op=ALU.add))
            step(lambda: vector.tensor_tensor(
                out=t16v, in0=t16v, in1=t64v[:, :, 32:48], op=ALU.add))
            step(lambda: vector.tensor_tensor(
                out=t16v, in0=t16v, in1=t64v[:, :, 48:64], op=ALU.add))
            step(lambda: vector.tensor_tensor(
                out=t4v, in0=t16v[:, :, 0:4], in1=t16v[:, :, 4:8], op=ALU.add))
            step(lambda: vector.tensor_tensor(
                out=t4v, in0=t4v, in1=t16v[:, :, 8:12], op=ALU.add))
            step(lambda: vector.tensor_tensor(
                out=t4v, in0=t4v, in1=t16v[:, :, 12:16], op=ALU.add))
            step(lambda: vector.tensor_tensor(
                out=t2v, in0=t4v[:, :, 0:2], in1=t4v[:, :, 2:4], op=ALU.add))
            step(lambda: vector.tensor_tensor(
                out=s_sb[:],
                in0=t2v[:, :, 0:1].rearrange("p r one -> p (r one)"),
                in1=t2v[:, :, 1:2].rearrange("p r one -> p (r one)"),
                op=ALU.add))
            # ---- fused broadcast * sign, per chunk ----
            vector.wait_ge(sm_sign, 16)
            for c in range(4):
                rc, ro = RCH[c], RCH_OFF[c]
                vector.tensor_tensor(
                    out=y_sb[:, ro * C : (ro + rc) * C].rearrange(
                        "p (r c) -> p r c", c=C
                    ),
                    in0=s_sb[:, ro : ro + rc].to_broadcast([P, rc, C]),
                    in1=sign_sb[:, ro * C : (ro + rc) * C].rearrange(
                        "p (r c) -> p r c", c=C
                    ),
                    op=ALU.mult,
                ).then_inc(sm_mult, 1)

    return nc


def _prep(src_idx, dst_idx):
    """Host index prep: per-core sign vectors + unshard positions."""
    src_flat = np.ascontiguousarray(src_idx).reshape(-1).astype(np.int64)
    dst_flat = np.ascontiguousarray(dst_idx).reshape(-1).astype(np.int64)
    n_ep = src_flat.shape[0]
    node_all = np.concatenate([src_flat, dst_flat])
    pos_all = np.concatenate([np.arange(n_ep, dtype=np.int64)] * 2)
    sgn_all = np.concatenate(
        [np.full(n_ep, INV_SQ, np.float32), np.full(n_ep, -INV_SQ, np.float32)]
    )

    per_core = []
    for i in range(NCORES):
        base = i * NPC
        m = (node_all >= base) & (node_all < base + NPC)
        loc = (node_all[m] - base).astype(np.int64)
        pos = pos_all[m]
        sgn = sgn_all[m]
        order = np.argsort(loc, kind="stable")
        loc, pos, sgn = loc[order], pos[order], sgn[order]
        cnt = np.bincount(loc, minlength=NPC)
        assert cnt.max() <= C, f"core {i}: max refs/node {cnt.max()} > C={C}"
        starts = np.zeros(NPC, np.int64)
        np.cumsum(cnt[:-1], out=starts[1:])
        within = np.arange(len(loc)) - starts[loc]
        slot = loc * C + within
        sign_of_slot = np.zeros(NPC * C, np.float32)
        sign_of_slot[slot] = sgn
        pos_of_slot = np.full(NPC * C, -1, np.int64)
        pos_of_slot[slot] = pos
        per_core.append(dict(sign_of_slot=sign_of_slot, pos_of_slot=pos_of_slot))
    return per_core


def kernel(node_embeds, src_idx, dst_idx):
    import ml_dtypes

    node_embeds = np.asarray(node_embeds, dtype=np.float32)
    T, E = np.asarray(src_idx).shape
    per_core = _prep(src_idx, dst_idx)

    emb_pad = np.zeros((NCORES * NPC, D), np.float32)
    emb_pad[:N_NODES] = node_embeds
    emb_bf = emb_pad.astype(ml_dtypes.bfloat16)

    nc = _build_nc()
    in_maps = []
    for i in range(NCORES):
        pc = per_core[i]
        in_maps.append(
            {
                "emb": np.ascontiguousarray(
                    emb_bf[i * NPC : (i + 1) * NPC].reshape(P, NPC)
                ),
                "sign_in": pc["sign_of_slot"]
                .astype(ml_dtypes.bfloat16)
                .reshape(P, R * C),
            }
        )
    res = run_bass_kernel_spmd(nc, in_maps, list(range(NCORES)))

    out_flat = np.zeros(T * E, np.float64)
    for i in range(NCORES):
        pc = per_core[i]
        vals = np.asarray(res.results[i]["y"], dtype=np.float32).reshape(-1)
        valid = pc["pos_of_slot"] >= 0
        np.add.at(out_flat, pc["pos_of_slot"][valid], vals[valid])
    return out_flat.reshape(T, E).astype(np.float32)


# revision 38
# speedup vs baseline: 1.3934x; 1.3934x over previous
"""Trainium2 Bass kernel for nn_DotPred (gnn_message_passing).

score[t, e] = sum_d (x[src] - x[dst]) / sqrt(D)
            = (rowsum(x)[src] - rowsum(x)[dst]) / sqrt(D)

Strategy (8 NeuronCores, SPMD): shard the NODE table across cores; each
endpoint reference (a src or dst occurrence of an edge) is handled by the
core owning that node. Each core emits one signed, scaled contribution per
reference; the host unshards with a scatter-add (each edge position gets
its +src and -dst contributions, possibly from different cores) —
mirroring the all-reduce combine in the sharding hint.

Per core (NPC = 12544 nodes, laid node-rank j = r*128 + p, ranks sorted by
descending reference count on the host):
- Phase 1: rowsum s[n] = sum_d x[n, d] from a bf16 node-slice laid
  [128, 98*128], via one 2x bf16 add level, a second add level, and a
  tensor_reduce per r-chunk on DVE.
- Phase 2 (bucket broadcast): node at (p, r) owns a bucket of cap_b slots
  where b is r's block; ref c of the node lives at y[p, c, r] within the
  block region. Count-sorted ranks make per-block capacities tight.
  y[p, c, r] = s[p, r] * sign[p, c, r] is one fused DVE tensor_tensor
  multiply per block: in0 = s broadcast along the MIDDLE axis with a
  stride-0 AP (keeps the packed last dim => 2x mode), in1 = host sign in
  {+-1/sqrt(D), 0}. No data-dependent addressing on device at all.
"""
import math
from contextlib import ExitStack

import numpy as np

import concourse.bass as bass
import concourse.mybir as mybir
from concourse.bass_utils import run_bass_kernel_spmd

P = 128
D = 128
R = 98                  # s-table columns per partition
NPC = P * R             # nodes per core = 12544
NCORES = 8
INV_SQ = 1.0 / math.sqrt(float(D))
N_NODES = 100000

# mult/sign/y blocks (count-sorted capacity tiers), sum = 98
RB = [7, 21, 21, 21, 21, 7]
RBOFF = [0, 7, 28, 49, 70, 91]
NB = len(RB)
# which tree chunks each block's r-range needs (cumulative reduce count)
BLK_RED = [1, 2, 3, 4, 5, 6]
# mult placement: True -> Pool engine
BLK_POOL = [False, False, True, True, False, False]
# tree/emb-load chunks (decoupled from blocks), sum = 98
TC = [14, 21, 21, 21, 14, 7]
TCOFF = [0, 14, 35, 56, 77, 91]
NCH = len(TC)

F32 = mybir.dt.float32
BF16 = mybir.dt.bfloat16
ALU = mybir.AluOpType
AX = mybir.AxisListType


def _build_nc(caps):
    tot = sum(c * rb for c, rb in zip(caps, RB))  # sign/y cols per partition
    yoff = np.concatenate([[0], np.cumsum([c * rb for c, rb in zip(caps, RB)])])

    nc = bass.Bass()
    emb = nc.dram_tensor("emb", [P, NPC], BF16, kind="ExternalInput")
    sign_in = nc.dram_tensor("sign_in", [P, tot], BF16, kind="ExternalInput")
    y = nc.dram_tensor("y", [P, tot], BF16, kind="ExternalOutput")

    es = ExitStack()
    with es:
        embb = es.enter_context(nc.sbuf_tensor([P, R * D], BF16))
        t64 = es.enter_context(nc.sbuf_tensor([P, R * 64], BF16))
        t32 = es.enter_context(nc.sbuf_tensor([P, R * 32], BF16))
        s_sb = es.enter_context(nc.sbuf_tensor([P, R], BF16))
        sign_sb = es.enter_context(nc.sbuf_tensor([P, tot], BF16))
        y_sb = es.enter_context(nc.sbuf_tensor([P, tot], BF16))

        sm_emb_l = [
            es.enter_context(nc.semaphore(name=f"sm_emb{c}")) for c in range(NCH)
        ]
        sm_sign_l = [
            es.enter_context(nc.semaphore(name=f"sm_sign{b}")) for b in range(NB)
        ]
        sm_tree = es.enter_context(nc.semaphore())
        sm_l2 = es.enter_context(nc.semaphore())
        sm_vch = es.enter_context(nc.semaphore())
        sm_mult_l = [
            es.enter_context(nc.semaphore(name=f"sm_mult{b}")) for b in range(NB)
        ]
        sm_y = es.enter_context(nc.semaphore())
        t64v = t64[:].rearrange("p (r d) -> p r d", d=64)
        t32v = t32[:].rearrange("p (r d) -> p r d", d=32)
        t64v_g = t64[:].rearrange("p (r d) -> p r d", d=64)
        t32v_g = t32[:].rearrange("p (r d) -> p r d", d=32)
        ev = embb[:].rearrange("p (r d) -> p r d", d=D)
        block = es.enter_context(nc.Block())

        @block.sync
        def _(sync):
            for c in range(NCH):
                sync.dma_start(
                    out=embb[:, TCOFF[c] * D : (TCOFF[c] + TC[c]) * D],
                    in_=emb[:, TCOFF[c] * D : (TCOFF[c] + TC[c]) * D],
                ).then_inc(sm_emb_l[c], 16)
            for b in range(NB):
                sync.dma_start(
                    out=sign_sb[:, yoff[b] : yoff[b + 1]],
                    in_=sign_in[:, yoff[b] : yoff[b + 1]],
                ).then_inc(sm_sign_l[b], 16)
            for b in range(NB):
                sync.wait_ge(sm_mult_l[b], 1)
                sync.dma_start(
                    out=y[:, yoff[b] : yoff[b + 1]],
                    in_=y_sb[:, yoff[b] : yoff[b + 1]],
                ).then_inc(sm_y, 16)

        def mult_op(eng, b):
            eng.wait_ge(sm_sign_l[b], 16)
            eng.wait_ge(sm_vch, BLK_RED[b])
            eng.tensor_tensor(
                out=y_sb[:, yoff[b] : yoff[b + 1]].rearrange(
                    "p (c r) -> p c r", r=RB[b]
                ),
                in0=s_sb[:, None, RBOFF[b] : RBOFF[b] + RB[b]].to_broadcast(
                    [P, caps[b], RB[b]]
                ),
                in1=sign_sb[:, yoff[b] : yoff[b + 1]].rearrange(
                    "p (c r) -> p c r", r=RB[b]
                ),
                op=ALU.mult,
            ).then_inc(sm_mult_l[b], 1)

        @block.gpsimd
        def _(gpsimd):
            for c in range(NCH):
                rs = slice(TCOFF[c], TCOFF[c] + TC[c])
                gpsimd.wait_ge(sm_tree, c + 1)
                gpsimd.tensor_tensor(
                    out=t32v_g[:, rs], in0=t64v_g[:, rs, 0:32],
                    in1=t64v_g[:, rs, 32:64], op=ALU.add,
                ).then_inc(sm_l2, 1)

        @block.vector
        def _(vector):
            vch = [0]

            def step(emit):
                emit().then_inc(sm_vch, 1)
                vch[0] += 1
                vector.wait_ge(sm_vch, vch[0])

            def lvl1(c):
                rs = slice(TCOFF[c], TCOFF[c] + TC[c])
                vector.wait_ge(sm_emb_l[c], 16)
                vector.tensor_tensor(
                    out=t64v[:, rs],
                    in0=ev[:, rs, 0:64],
                    in1=ev[:, rs, 64:128],
                    op=ALU.add,
                ).then_inc(sm_tree, 1)

            def red(c):
                rs = slice(TCOFF[c], TCOFF[c] + TC[c])
                vector.wait_ge(sm_l2, c + 1)
                with nc.allow_low_precision(reason="bf16 rowsum; tol 2e-2"):
                    step(lambda: vector.tensor_reduce(
                        out=s_sb[:, rs], in_=t32v[:, rs], op=ALU.add, axis=AX.X))

            lvl1(0)
            lvl1(1)
            red(0)
            lvl1(2)
            red(1)
            lvl1(3)
            red(2)
            mult_op(vector, 0)
            lvl1(4)
            red(3)
            mult_op(vector, 1)
            lvl1(5)
            red(4)
            mult_op(vector, 2)
            mult_op(vector, 3)
            mult_op(vector, 4)
            red(5)
            mult_op(vector, 5)

    return nc


def _prep(src_idx, dst_idx):
    """Host index prep: per-core count-sorted ranks, block caps, sign/pos."""
    src_flat = np.ascontiguousarray(src_idx).reshape(-1).astype(np.int64)
    dst_flat = np.ascontiguousarray(dst_idx).reshape(-1).astype(np.int64)
    n_ep = src_flat.shape[0]
    node_all = np.concatenate([src_flat, dst_flat])
    pos_all = np.concatenate([np.arange(n_ep, dtype=np.int64)] * 2)
    sgn_all = np.concatenate(
        [np.full(n_ep, INV_SQ, np.float32), np.full(n_ep, -INV_SQ, np.float32)]
    )

    cores = []
    blockmax = np.zeros((NCORES, NB), np.int64)
    for i in range(NCORES):
        base = i * NPC
        m = (node_all >= base) & (node_all < base + NPC)
        loc = (node_all[m] - base).astype(np.int64)
        pos = pos_all[m]
        sgn = sgn_all[m]
        cnt = np.bincount(loc, minlength=NPC)
        perm = np.argsort(-cnt, kind="stable")     # rank j -> local node id
        rank_of = np.empty(NPC, np.int64)
        rank_of[perm] = np.arange(NPC)
        scnt = cnt[perm]                           # counts by rank
        for b in range(NB):
            blockmax[i, b] = scnt[RBOFF[b] * P : (RBOFF[b] + RB[b]) * P].max()
        cores.append(dict(loc=loc, pos=pos, sgn=sgn, cnt=cnt, perm=perm,
                          rank_of=rank_of))
    caps = [int(blockmax[:, b].max()) for b in range(NB)]
    tot = sum(c * rb for c, rb in zip(caps, RB))
    yoff = np.concatenate([[0], np.cumsum([c * rb for c, rb in zip(caps, RB)])])
    # block id and within-block r for each rank's r = rank // P
    rblk = np.zeros(R, np.int64)
    for b in range(NB):
        rblk[RBOFF[b] : RBOFF[b] + RB[b]] = b

    per_core = []
    for i in range(NCORES):
        cc = cores[i]
        loc, pos, sgn = cc["loc"], cc["pos"], cc["sgn"]
        rank = cc["rank_of"][loc]                  # rank of each reference
        order = np.argsort(rank, kind="stable")
        rank, pos, sgn = rank[order], pos[order], sgn[order]
        scnt = cc["cnt"][cc["perm"]]
        starts = np.zeros(NPC, np.int64)
        np.cumsum(scnt[:-1], out=starts[1:])
        within = np.arange(len(rank)) - starts[rank]
        # rank j = r*128 + p ; slot = p*tot + yoff[b] + c*RB[b] + (r - RBOFF[b])
        r = rank // P
        p = rank % P
        b = rblk[r]
        slot = (
            p * tot
            + yoff[b]
            + within * np.array(RB)[b]
            + (r - np.array(RBOFF)[b])
        )
        sign_of_slot = np.zeros(P * tot, np.float32)
        sign_of_slot[slot] = sgn
        pos_of_slot = np.full(P * tot, -1, np.int64)
        pos_of_slot[slot] = pos
        per_core.append(
            dict(sign_of_slot=sign_of_slot, pos_of_slot=pos_of_slot,
                 perm=cc["perm"])
        )
    return per_core, caps, tot


def kernel(node_embeds, src_idx, dst_idx):
    import ml_dtypes

    node_embeds = np.asarray(node_embeds, dtype=np.float32)
    T, E = np.asarray(src_idx).shape
    per_core, caps, tot = _prep(src_idx, dst_idx)

    emb_pad = np.zeros((NCORES * NPC, D), np.float32)
    emb_pad[:N_NODES] = node_embeds
    emb_bf = emb_pad.astype(ml_dtypes.bfloat16)

    nc = _build_nc(caps)
    in_maps = []
    for i in range(NCORES):
        pc = per_core[i]
        core_emb = emb_bf[i * NPC : (i + 1) * NPC]
        # emb[p, r*D + d] = embeds[perm[r*128 + p], d]
        emb_lay = np.ascontiguousarray(
            core_emb[pc["perm"]].reshape(R, P, D).transpose(1, 0, 2)
        ).reshape(P, NPC)
        in_maps.append(
            {
                "emb": emb_lay,
                "sign_in": pc["sign_of_slot"]
                .astype(ml_dtypes.bfloat16)
                .reshape(P, tot),
            }
        )
    res = run_bass_kernel_spmd(nc, in_maps, list(range(NCORES)))

    out_flat = np.zeros(T * E, np.float64)
    for i in range(NCORES):
        pc = per_core[i]
        vals = np.asarray(res.results[i]["y"], dtype=np.float32).reshape(-1)
        valid = pc["pos_of_slot"] >= 0
        np.add.at(out_flat, pc["pos_of_slot"][valid], vals[valid])
    return out_flat.reshape(T, E).astype(np.float32)


# revision 47
# speedup vs baseline: 1.4194x; 1.0186x over previous
"""Trainium2 Bass kernel for nn_DotPred (gnn_message_passing).

score[t, e] = sum_d (x[src] - x[dst]) / sqrt(D)
            = (rowsum(x)[src] - rowsum(x)[dst]) / sqrt(D)

Strategy (8 NeuronCores, SPMD): shard the NODE table across cores; each
endpoint reference (a src or dst occurrence of an edge) is handled by the
core owning that node. Each core emits one signed, scaled contribution per
reference; the host unshards with a scatter-add (each edge position gets
its +src and -dst contributions, possibly from different cores) —
mirroring the all-reduce combine in the sharding hint.

Per core (NPC = 12544 nodes, laid node-rank j = r*128 + p, ranks sorted by
descending reference count on the host):
- Phase 1: rowsum s[n] = sum_d x[n, d] from a bf16 node-slice laid
  [128, 98*128], via one 2x bf16 add level, a second add level, and a
  tensor_reduce per r-chunk on DVE.
- Phase 2 (bucket broadcast): node at (p, r) owns a bucket of cap_b slots
  where b is r's block; ref c of the node lives at y[p, c, r] within the
  block region. Count-sorted ranks make per-block capacities tight.
  y[p, c, r] = s[p, r] * sign[p, c, r] is one fused DVE tensor_tensor
  multiply per block: in0 = s broadcast along the MIDDLE axis with a
  stride-0 AP (keeps the packed last dim => 2x mode), in1 = host sign in
  {+-1/sqrt(D), 0}. No data-dependent addressing on device at all.
"""
import math
from contextlib import ExitStack

import numpy as np

import concourse.bass as bass
import concourse.mybir as mybir
from concourse.bass_utils import run_bass_kernel_spmd

P = 128
D = 128
R = 98                  # s-table columns per partition
NPC = P * R             # nodes per core = 12544
NCORES = 8
INV_SQ = 1.0 / math.sqrt(float(D))
N_NODES = 100000

# mult/sign/y blocks (count-sorted capacity tiers), sum = 98
RB = [7, 21, 21, 21, 21, 7]
RBOFF = [0, 7, 28, 49, 70, 91]
NB = len(RB)
# which tree chunks each block's r-range needs (cumulative reduce count)
BLK_RED = [1, 2, 3, 4, 5, 6]
# tree/emb-load chunks (decoupled from blocks), sum = 98
TC = [14, 21, 21, 21, 14, 7]
TCOFF = [0, 14, 35, 56, 77, 91]
NCH = len(TC)

F32 = mybir.dt.float32
BF16 = mybir.dt.bfloat16
ALU = mybir.AluOpType
AX = mybir.AxisListType


def _build_nc(caps):
    tot = sum(c * rb for c, rb in zip(caps, RB))  # sign/y cols per partition
    yoff = np.concatenate([[0], np.cumsum([c * rb for c, rb in zip(caps, RB)])])

    nc = bass.Bass()
    emb = nc.dram_tensor("emb", [P, NPC], BF16, kind="ExternalInput")
    sign_in = nc.dram_tensor("sign_in", [P, tot], BF16, kind="ExternalInput")
    y = nc.dram_tensor("y", [P, tot], BF16, kind="ExternalOutput")

    es = ExitStack()
    with es:
        t64 = es.enter_context(nc.sbuf_tensor([P, R * 64], BF16))
        t32 = es.enter_context(nc.sbuf_tensor([P, R * 32], BF16))
        s_sb = es.enter_context(nc.sbuf_tensor([P, R], BF16))
        sign_sb = es.enter_context(nc.sbuf_tensor([P, tot], BF16))
        y_sb = es.enter_context(nc.sbuf_tensor([P, tot], BF16))
        embb = es.enter_context(nc.sbuf_tensor([P, R * D], BF16))

        sm_emb_l = [
            es.enter_context(nc.semaphore(name=f"sm_emb{c}")) for c in range(NCH)
        ]
        sm_sign_l = [
            es.enter_context(nc.semaphore(name=f"sm_sign{b}")) for b in range(NB)
        ]
        sm_tree = es.enter_context(nc.semaphore())
        sm_l2 = es.enter_context(nc.semaphore())
        sm_vch = es.enter_context(nc.semaphore())
        sm_mult_l = [
            es.enter_context(nc.semaphore(name=f"sm_mult{b}")) for b in range(NB)
        ]
        sm_y = es.enter_context(nc.semaphore())
        t64v = t64[:].rearrange("p (r d) -> p r d", d=64)
        t32v = t32[:].rearrange("p (r d) -> p r d", d=32)
        t64v_g = t64[:].rearrange("p (r d) -> p r d", d=64)
        t32v_g = t32[:].rearrange("p (r d) -> p r d", d=32)
        block = es.enter_context(nc.Block())

        @block.sync
        def _(sync):
            for c in range(NCH):
                sync.dma_start(
                    out=embb[:, TCOFF[c] * D : (TCOFF[c] + TC[c]) * D],
                    in_=emb[:, TCOFF[c] * D : (TCOFF[c] + TC[c]) * D],
                ).then_inc(sm_emb_l[c], 16)
            for b in range(NB):
                sync.dma_start(
                    out=sign_sb[:, yoff[b] : yoff[b + 1]],
                    in_=sign_in[:, yoff[b] : yoff[b + 1]],
                ).then_inc(sm_sign_l[b], 16)
            for b in [0, 2, 4]:
                sync.wait_ge(sm_mult_l[b], 1)
                sync.dma_start(
                    out=y[:, yoff[b] : yoff[b + 1]],
                    in_=y_sb[:, yoff[b] : yoff[b + 1]],
                ).then_inc(sm_y, 16)

        @block.scalar
        def _(scalar):
            for b in [1, 3, 5]:
                scalar.wait_ge(sm_mult_l[b], 1)
                scalar.dma_start(
                    out=y[:, yoff[b] : yoff[b + 1]],
                    in_=y_sb[:, yoff[b] : yoff[b + 1]],
                ).then_inc(sm_y, 16)

        def mult_op(eng, b):
            eng.wait_ge(sm_sign_l[b], 16)
            eng.wait_ge(sm_vch, BLK_RED[b])
            eng.tensor_tensor(
                out=y_sb[:, yoff[b] : yoff[b + 1]].rearrange(
                    "p (c r) -> p c r", r=RB[b]
                ),
                in0=s_sb[:, None, RBOFF[b] : RBOFF[b] + RB[b]].to_broadcast(
                    [P, caps[b], RB[b]]
                ),
                in1=sign_sb[:, yoff[b] : yoff[b + 1]].rearrange(
                    "p (c r) -> p c r", r=RB[b]
                ),
                op=ALU.mult,
            ).then_inc(sm_mult_l[b], 1)

        @block.gpsimd
        def _(gpsimd):
            for c in range(NCH):
                rs = slice(TCOFF[c], TCOFF[c] + TC[c])
                gpsimd.wait_ge(sm_tree, c + 1)
                gpsimd.tensor_tensor(
                    out=t32v_g[:, rs], in0=t64v_g[:, rs, 0:32],
                    in1=t64v_g[:, rs, 32:64], op=ALU.add,
                ).then_inc(sm_l2, 1)

        @block.vector
        def _(vector):
            vch = [0]

            def step(emit):
                emit().then_inc(sm_vch, 1)
                vch[0] += 1
                vector.wait_ge(sm_vch, vch[0])

            def lvl1(c):
                rs = slice(TCOFF[c], TCOFF[c] + TC[c])
                vector.wait_ge(sm_emb_l[c], 16)
                vector.tensor_tensor(
                    out=t64v[:, rs],
                    in0=ev[:, rs, 0:64],
                    in1=ev[:, rs, 64:128],
                    op=ALU.add,
                ).then_inc(sm_tree, 1)

            def red(c):
                rs = slice(TCOFF[c], TCOFF[c] + TC[c])
                vector.wait_ge(sm_l2, c + 1)
                with nc.allow_low_precision(reason="bf16 rowsum; tol 2e-2"):
                    step(lambda: vector.tensor_reduce(
                        out=s_sb[:, rs], in_=t32v[:, rs], op=ALU.add, axis=AX.X))

            ev = embb[:].rearrange("p (r d) -> p r d", d=D)


            lvl1(0)
            lvl1(1)
            red(0)
            lvl1(2)
            red(1)
            lvl1(3)
            red(2)
            mult_op(vector, 0)
            lvl1(4)
            red(3)
            mult_op(vector, 1)
            lvl1(5)
            red(4)
            mult_op(vector, 2)
            mult_op(vector, 3)
            mult_op(vector, 4)
            red(5)
            mult_op(vector, 5)

    return nc


def _prep(src_idx, dst_idx):
    """Host index prep: per-core count-sorted ranks, block caps, sign/pos."""
    src_flat = np.ascontiguousarray(src_idx).reshape(-1).astype(np.int64)
    dst_flat = np.ascontiguousarray(dst_idx).reshape(-1).astype(np.int64)
    n_ep = src_flat.shape[0]
    node_all = np.concatenate([src_flat, dst_flat])
    pos_all = np.concatenate([np.arange(n_ep, dtype=np.int64)] * 2)
    sgn_all = np.concatenate(
        [np.full(n_ep, INV_SQ, np.float32), np.full(n_ep, -INV_SQ, np.float32)]
    )

    cores = []
    blockmax = np.zeros((NCORES, NB), np.int64)
    for i in range(NCORES):
        base = i * NPC
        m = (node_all >= base) & (node_all < base + NPC)
        loc = (node_all[m] - base).astype(np.int64)
        pos = pos_all[m]
        sgn = sgn_all[m]
        cnt = np.bincount(loc, minlength=NPC)
        perm = np.argsort(-cnt, kind="stable")     # rank j -> local node id
        rank_of = np.empty(NPC, np.int64)
        rank_of[perm] = np.arange(NPC)
        scnt = cnt[perm]                           # counts by rank
        for b in range(NB):
            blockmax[i, b] = scnt[RBOFF[b] * P : (RBOFF[b] + RB[b]) * P].max()
        cores.append(dict(loc=loc, pos=pos, sgn=sgn, cnt=cnt, perm=perm,
                          rank_of=rank_of))
    caps = [int(blockmax[:, b].max()) for b in range(NB)]
    tot = sum(c * rb for c, rb in zip(caps, RB))
    yoff = np.concatenate([[0], np.cumsum([c * rb for c, rb in zip(caps, RB)])])
    # block id and within-block r for each rank's r = rank // P
    rblk = np.zeros(R, np.int64)
    for b in range(NB):
        rblk[RBOFF[b] : RBOFF[b] + RB[b]] = b

    per_core = []
    for i in range(NCORES):
        cc = cores[i]
        loc, pos, sgn = cc["loc"], cc["pos"], cc["sgn"]
        rank = cc["rank_of"][loc]                  # rank of each reference
        order = np.argsort(rank, kind="stable")
        rank, pos, sgn = rank[order], pos[order], sgn[order]
        scnt = cc["cnt"][cc["perm"]]
        starts = np.zeros(NPC, np.int64)
        np.cumsum(scnt[:-1], out=starts[1:])
        within = np.arange(len(rank)) - starts[rank]
        # rank j = r*128 + p ; slot = p*tot + yoff[b] + c*RB[b] + (r - RBOFF[b])
        r = rank // P
        p = rank % P
        b = rblk[r]
        slot = (
            p * tot
            + yoff[b]
            + within * np.array(RB)[b]
            + (r - np.array(RBOFF)[b])
        )
        sign_of_slot = np.zeros(P * tot, np.float32)
        sign_of_slot[slot] = sgn
        pos_of_slot = np.full(P * tot, -1, np.int64)
        pos_of_slot[slot] = pos
        per_core.append(
            dict(sign_of_slot=sign_of_slot, pos_of_slot=pos_of_slot,
                 perm=cc["perm"])
        )
    return per_core, caps, tot


def kernel(node_embeds, src_idx, dst_idx):
    import ml_dtypes

    node_embeds = np.asarray(node_embeds, dtype=np.float32)
    T, E = np.asarray(src_idx).shape
    per_core, caps, tot = _prep(src_idx, dst_idx)

    emb_pad = np.zeros((NCORES * NPC, D), np.float32)
    emb_pad[:N_NODES] = node_embeds
    emb_bf = emb_pad.astype(ml_dtypes.bfloat16)

    nc = _build_nc(caps)
    in_maps = []
    for i in range(NCORES):
        pc = per_core[i]
        core_emb = emb_bf[i * NPC : (i + 1) * NPC]
        # emb[p, r*D + d] = embeds[perm[r*128 + p], d]
        emb_lay = np.ascontiguousarray(
            core_emb[pc["perm"]].reshape(R, P, D).transpose(1, 0, 2)
        ).reshape(P, NPC)
        in_maps.append(
            {
                "emb": emb_lay,
                "sign_in": pc["sign_of_slot"]
                .astype(ml_dtypes.bfloat16)
                .reshape(P, tot),
            }
        )
    res = run_bass_kernel_spmd(nc, in_maps, list(range(NCORES)))

    out_flat = np.zeros(T * E, np.float64)
    for i in range(NCORES):
        pc = per_core[i]
        vals = np.asarray(res.results[i]["y"], dtype=np.float32).reshape(-1)
        valid = pc["pos_of_slot"] >= 0
        np.add.at(out_flat, pc["pos_of_slot"][valid], vals[valid])
    return out_flat.reshape(T, E).astype(np.float32)


# revision 57
# speedup vs baseline: 1.4673x; 1.0337x over previous
"""Trainium2 Bass kernel for nn_DotPred (gnn_message_passing).

score[t, e] = sum_d (x[src] - x[dst]) / sqrt(D)
            = (rowsum(x)[src] - rowsum(x)[dst]) / sqrt(D)

Strategy (8 NeuronCores, SPMD): shard the NODE table across cores; each
endpoint reference (a src or dst occurrence of an edge) is handled by the
core owning that node. Each core emits one signed, scaled contribution per
reference; the host unshards with a scatter-add (each edge position gets
its +src and -dst contributions, possibly from different cores) —
mirroring the all-reduce combine in the sharding hint.

Per core (NPC = 12544 nodes, laid node-rank j = r*128 + p, ranks sorted by
descending reference count on the host):
- Phase 1: rowsum s[n] = sum_d x[n, d] from a bf16 node-slice laid
  [128, 98*128], via one 2x bf16 add level, a second add level, and a
  tensor_reduce per r-chunk on DVE.
- Phase 2 (bucket broadcast): node at (p, r) owns a bucket of cap_b slots
  where b is r's block; ref c of the node lives at y[p, c, r] within the
  block region. Count-sorted ranks make per-block capacities tight.
  y[p, c, r] = s[p, r] * sign[p, c, r] is one fused DVE tensor_tensor
  multiply per block: in0 = s broadcast along the MIDDLE axis with a
  stride-0 AP (keeps the packed last dim => 2x mode), in1 = host sign in
  {+-1/sqrt(D), 0}. No data-dependent addressing on device at all.
"""
import math
from contextlib import ExitStack

import numpy as np

import concourse.bass as bass
import concourse.mybir as mybir
from concourse.bass_utils import run_bass_kernel_spmd

P = 128
D = 128
R = 98                  # s-table columns per partition
NPC = P * R             # nodes per core = 12544
NCORES = 8
INV_SQ = 1.0 / math.sqrt(float(D))
N_NODES = 100000

# mult/sign/y blocks (count-sorted capacity tiers), sum = 98
RB = [7, 21, 21, 21, 21, 7]
RBOFF = [0, 7, 28, 49, 70, 91]
NB = len(RB)
# which tree chunks each block's r-range needs (cumulative reduce count)
BLK_RED = [1, 2, 3, 4, 5, 6]
# tree/emb-load chunks (decoupled from blocks), sum = 98
TC = [14, 21, 21, 21, 14, 7]
TCOFF = [0, 14, 35, 56, 77, 91]
NCH = len(TC)

F32 = mybir.dt.float32
BF16 = mybir.dt.bfloat16
ALU = mybir.AluOpType
AX = mybir.AxisListType


def _build_nc(caps):
    tot = sum(c * rb for c, rb in zip(caps, RB))  # sign/y cols per partition
    yoff = np.concatenate([[0], np.cumsum([c * rb for c, rb in zip(caps, RB)])])

    nc = bass.Bass()
    emb = nc.dram_tensor("emb", [P, NPC], BF16, kind="ExternalInput")
    sign_in = nc.dram_tensor("sign_in", [P, tot], BF16, kind="ExternalInput")
    y = nc.dram_tensor("y", [P, tot], BF16, kind="ExternalOutput")

    es = ExitStack()
    with es:
        t64 = es.enter_context(nc.sbuf_tensor([P, R * 64], BF16))
        t32 = es.enter_context(nc.sbuf_tensor([P, R * 32], BF16))
        s_sb = es.enter_context(nc.sbuf_tensor([P, R], BF16))
        sign_sb = es.enter_context(nc.sbuf_tensor([P, tot], BF16))
        y_sb = es.enter_context(nc.sbuf_tensor([P, tot], BF16))
        embb = es.enter_context(nc.sbuf_tensor([P, R * D], BF16))

        sm_emb_l = [
            es.enter_context(nc.semaphore(name=f"sm_emb{c}")) for c in range(NCH)
        ]
        sm_sign_l = [
            es.enter_context(nc.semaphore(name=f"sm_sign{b}")) for b in range(NB)
        ]
        sm_tree = es.enter_context(nc.semaphore())
        sm_l2 = es.enter_context(nc.semaphore())
        sm_vch = es.enter_context(nc.semaphore())
        sm_mult_l = [
            es.enter_context(nc.semaphore(name=f"sm_mult{b}")) for b in range(NB)
        ]
        sm_y = es.enter_context(nc.semaphore())
        t64v = t64[:].rearrange("p (r d) -> p r d", d=64)
        t32v = t32[:].rearrange("p (r d) -> p r d", d=32)
        t64v_g = t64[:].rearrange("p (r d) -> p r d", d=64)
        t32v_g = t32[:].rearrange("p (r d) -> p r d", d=32)
        block = es.enter_context(nc.Block())

        @block.sync
        def _(sync):
            for c in range(NCH):
                sync.dma_start(
                    out=embb[:, TCOFF[c] * D : (TCOFF[c] + TC[c]) * D],
                    in_=emb[:, TCOFF[c] * D : (TCOFF[c] + TC[c]) * D],
                ).then_inc(sm_emb_l[c], 16)
            for b in range(NB):
                sync.dma_start(
                    out=sign_sb[:, yoff[b] : yoff[b + 1]],
                    in_=sign_in[:, yoff[b] : yoff[b + 1]],
                ).then_inc(sm_sign_l[b], 16)
            for b in [0, 3, 5]:
                sync.wait_ge(sm_mult_l[b], 1)
                sync.dma_start(
                    out=y[:, yoff[b] : yoff[b + 1]],
                    in_=y_sb[:, yoff[b] : yoff[b + 1]],
                ).then_inc(sm_y, 16)

        @block.scalar
        def _(scalar):
            for b in [1, 2, 4]:
                scalar.wait_ge(sm_mult_l[b], 1)
                scalar.dma_start(
                    out=y[:, yoff[b] : yoff[b + 1]],
                    in_=y_sb[:, yoff[b] : yoff[b + 1]],
                ).then_inc(sm_y, 16)

        def mult_op(eng, b):
            eng.wait_ge(sm_sign_l[b], 16)
            eng.wait_ge(sm_vch, BLK_RED[b])
            eng.tensor_tensor(
                out=y_sb[:, yoff[b] : yoff[b + 1]].rearrange(
                    "p (c r) -> p c r", r=RB[b]
                ),
                in0=s_sb[:, None, RBOFF[b] : RBOFF[b] + RB[b]].to_broadcast(
                    [P, caps[b], RB[b]]
                ),
                in1=sign_sb[:, yoff[b] : yoff[b + 1]].rearrange(
                    "p (c r) -> p c r", r=RB[b]
                ),
                op=ALU.mult,
            ).then_inc(sm_mult_l[b], 1)

        @block.gpsimd
        def _(gpsimd):
            for c in range(NCH):
                rs = slice(TCOFF[c], TCOFF[c] + TC[c])
                gpsimd.wait_ge(sm_tree, c + 1)
                gpsimd.tensor_tensor(
                    out=t32v_g[:, rs], in0=t64v_g[:, rs, 0:32],
                    in1=t64v_g[:, rs, 32:64], op=ALU.add,
                ).then_inc(sm_l2, 1)

        @block.vector
        def _(vector):
            vch = [0]

            def step(emit):
                emit().then_inc(sm_vch, 1)
                vch[0] += 1
                vector.wait_ge(sm_vch, vch[0])

            def lvl1(c):
                rs = slice(TCOFF[c], TCOFF[c] + TC[c])
                vector.wait_ge(sm_emb_l[c], 16)
                vector.tensor_tensor(
                    out=t64v[:, rs],
                    in0=ev[:, rs, 0:64],
                    in1=ev[:, rs, 64:128],
                    op=ALU.add,
                ).then_inc(sm_tree, 1)

            def red(c):
                rs = slice(TCOFF[c], TCOFF[c] + TC[c])
                vector.wait_ge(sm_l2, c + 1)
                with nc.allow_low_precision(reason="bf16 rowsum; tol 2e-2"):
                    step(lambda: vector.tensor_reduce(
                        out=s_sb[:, rs], in_=t32v[:, rs], op=ALU.add, axis=AX.X))

            ev = embb[:].rearrange("p (r d) -> p r d", d=D)


            lvl1(0)
            lvl1(1)
            red(0)
            lvl1(2)
            red(1)
            lvl1(3)
            red(2)
            lvl1(4)
            mult_op(vector, 0)
            red(3)
            lvl1(5)
            mult_op(vector, 1)
            red(4)
            mult_op(vector, 2)
            mult_op(vector, 3)
            mult_op(vector, 4)
            red(5)
            mult_op(vector, 5)

    return nc


def _prep(src_idx, dst_idx):
    """Host index prep: per-core count-sorted ranks, block caps, sign/pos."""
    src_flat = np.ascontiguousarray(src_idx).reshape(-1).astype(np.int64)
    dst_flat = np.ascontiguousarray(dst_idx).reshape(-1).astype(np.int64)
    n_ep = src_flat.shape[0]
    node_all = np.concatenate([src_flat, dst_flat])
    pos_all = np.concatenate([np.arange(n_ep, dtype=np.int64)] * 2)
    sgn_all = np.concatenate(
        [np.full(n_ep, INV_SQ, np.float32), np.full(n_ep, -INV_SQ, np.float32)]
    )

    cores = []
    blockmax = np.zeros((NCORES, NB), np.int64)
    for i in range(NCORES):
        base = i * NPC
        m = (node_all >= base) & (node_all < base + NPC)
        loc = (node_all[m] - base).astype(np.int64)
        pos = pos_all[m]
        sgn = sgn_all[m]
        cnt = np.bincount(loc, minlength=NPC)
        perm = np.argsort(-cnt, kind="stable")     # rank j -> local node id
        rank_of = np.empty(NPC, np.int64)
        rank_of[perm] = np.arange(NPC)
        scnt = cnt[perm]                           # counts by rank
        for b in range(NB):
            blockmax[i, b] = scnt[RBOFF[b] * P : (RBOFF[b] + RB[b]) * P].max()
        cores.append(dict(loc=loc, pos=pos, sgn=sgn, cnt=cnt, perm=perm,
                          rank_of=rank_of))
    caps = [int(blockmax[:, b].max()) for b in range(NB)]
    tot = sum(c * rb for c, rb in zip(caps, RB))
    yoff = np.concatenate([[0], np.cumsum([c * rb for c, rb in zip(caps, RB)])])
    # block id and within-block r for each rank's r = rank // P
    rblk = np.zeros(R, np.int64)
    for b in range(NB):
        rblk[RBOFF[b] : RBOFF[b] + RB[b]] = b

    per_core = []
    for i in range(NCORES):
        cc = cores[i]
        loc, pos, sgn = cc["loc"], cc["pos"], cc["sgn"]
        rank = cc["rank_of"][loc]                  # rank of each reference
        order = np.argsort(rank, kind="stable")
        rank, pos, sgn = rank[order], pos[order], sgn[order]
        scnt = cc["cnt"][cc["perm"]]
        starts = np.zeros(NPC, np.int64)
        np.cumsum(scnt[:-1], out=starts[1:])
        within = np.arange(len(rank)) - starts[rank]
        # rank j = r*128 + p ; slot = p*tot + yoff[b] + c*RB[b] + (r - RBOFF[b])
        r = rank // P
        p = rank % P
        b = rblk[r]
        slot = (
            p * tot
            + yoff[b]
            + within * np.array(RB)[b]
            + (r - np.array(RBOFF)[b])
        )
        sign_of_slot = np.zeros(P * tot, np.float32)
        sign_of_slot[slot] = sgn
        pos_of_slot = np.full(P * tot, -1, np.int64)
        pos_of_slot[slot] = pos
        per_core.append(
            dict(sign_of_slot=sign_of_slot, pos_of_slot=pos_of_slot,
                 perm=cc["perm"])
        )
    return per_core, caps, tot


def kernel(node_embeds, src_idx, dst_idx):
    import ml_dtypes

    node_embeds = np.asarray(node_embeds, dtype=np.float32)
    T, E = np.asarray(src_idx).shape
    per_core, caps, tot = _prep(src_idx, dst_idx)

    emb_pad = np.zeros((NCORES * NPC, D), np.float32)
    emb_pad[:N_NODES] = node_embeds
    emb_bf = emb_pad.astype(ml_dtypes.bfloat16)

    nc = _build_nc(caps)
    in_maps = []
    for i in range(NCORES):
        pc = per_core[i]
        core_emb = emb_bf[i * NPC : (i + 1) * NPC]
        # emb[p, r*D + d] = embeds[perm[r*128 + p], d]
        emb_lay = np.ascontiguousarray(
            core_emb[pc["perm"]].reshape(R, P, D).transpose(1, 0, 2)
        ).reshape(P, NPC)
        in_maps.append(
            {
                "emb": emb_lay,
                "sign_in": pc["sign_of_slot"]
                .astype(ml_dtypes.bfloat16)
                .reshape(P, tot),
            }
        )
    res = run_bass_kernel_spmd(nc, in_maps, list(range(NCORES)))

    out_flat = np.zeros(T * E, np.float64)
    for i in range(NCORES):
        pc = per_core[i]
        vals = np.asarray(res.results[i]["y"], dtype=np.float32).reshape(-1)
        valid = pc["pos_of_slot"] >= 0
        np.add.at(out_flat, pc["pos_of_slot"][valid], vals[valid])
    return out_flat.reshape(T, E).astype(np.float32)
